# revision 26
# baseline (speedup 1.0000x reference)
"""Fused multi-head attention block (qkv proj + softmax(QK^T)V + out proj)
for Trainium2, SPMD across 8 NeuronCores.

Sharding: 8 cores = 2 batches x 4 head-groups (4 heads/core, data parallel on
B, tensor parallel on heads). Each core computes its 4 heads end-to-end plus a
row-parallel slice of the output projection; the 4 partial y's per batch are
summed on the host (with b_proj added once).

v3 dataflow (per core, fp8-DoubleRow hi/lo decomposition):
  - QKV: x and the (host-x32-scaled) weights are shipped as fp8e4 hi/lo
    pairs; each projection accumulates three DoubleRow chains
    (xh*wh + xh*wl + xl*wh) in PSUM, then an affine pass applies
    (psum + 32*bias)/32. q/k come out [ch, T]; v comes out [T, ch] with a
    ones column per head (row sums -> softmax denominators).
  - scores: per head, ONE DoubleRow matmul computes kh.qh + kl.qh + kh.ql
    (K = 2x128: slot0 = [kh;kl]x[qh;qh], slot1 = [kh;0]x[ql;*]). The fp8
    hi/lo q/k operand tiles are built by DVE cast passes into staged
    [2-head, T] tiles plus SBUF->SBUF DMAs that place/duplicate the 64-row
    halves (issued on the ACT queue for a=0, SP for a=1).
  - softmax: P.T = exp(S.T * 0.125) out of PSUM, bf16; most steps on ACT
    (native Exp), 3 of 16 per block on DVE/Pool via a Schraudolph bit-trick
    (t = s*a + b -> int16 -> bitcast bf16). Normalization deferred via the
    ones column in v; no max subtraction needed at these score magnitudes.
  - PV: O.T_aug[65, i] += v_aug[jc].T @ P.T[jc], bf16 matmuls.
  - normalize: one [65,512] copy frees the PSUM bank, then recip/broadcast/
    multiply off SBUF -> attn.T bf16.
  - proj: split by K-half. The cc=0 half (a=0 heads) runs right after the
    a=0 norm and is staged to SBUF; the cc=1 half runs after the a=1 norm
    and a tensor_tensor add emits yT bf16 partials (keeps the tail short).
"""

from contextlib import ExitStack

import ml_dtypes
import numpy as np

import concourse.bass as bass
import concourse.mybir as mybir
import concourse.tile as tile
from concourse import bacc
from concourse.bass_utils import run_bass_kernel_spmd

F32 = mybir.dt.float32
BF16 = mybir.dt.bfloat16
FP8 = mybir.dt.float8e4
I16 = mybir.dt.int16
FT = mybir.ActivationFunctionType
OP = mybir.AluOpType
DR = mybir.MatmulPerfMode.DoubleRow

B, D = 2, 1024
H, HD = 16, 64
NCORES = 8
HPC = 4                # heads per core
CH = HPC * HD          # 256 q/k/v channels per core
P = 128
W_SCALE = 32.0         # host-side weight scale before fp8 hi/lo split
SCALE = 1.0 / float(np.sqrt(HD))
# Schraudolph bf16 exp: floor(s*EXA + EXB) as int16, bitcast bf16 ~= exp(s*SCALE)
EXA = 128.0 * float(np.log2(np.e)) * SCALE
EXB = 127.0 * 128.0 - 7.5 + 0.5   # c=-7.5 calibrated; +0.5 turns floor into round
OFF_STEPS = {5: "dve", 11: "dve"}   # 2-op schraudolph (fused PSUM->int16 is broken on HW)


def build_body(tc, ctx, T, xh, xl, wqkhl, wvhl, bqk, bv, wpT, yT,
               repeat=1, lag=8):
    nc = tc.nc
    TI = T // 512          # 512-wide i (query) chunks
    TJ = T // P            # 128-wide j (key) chunks
    KC = 4                 # DoubleRow contraction chunks (K=1024 as 4x256)
    MC = D // P            # 8 output-row chunks

    xh_r = xh.ap().rearrange("(c s p) t -> p c s t", p=P, s=2)
    xl_r = xl.ap().rearrange("(c s p) t -> p c s t", p=P, s=2)
    wqk_r = wqkhl.ap().rearrange("(c s p) h m -> p c s h m", p=P, s=2)
    wv_r = wvhl.ap().rearrange("(c s p) h m -> p c s h m", p=P, s=2)
    wp_r = wpT.ap().rearrange("(cc p) m -> p cc m", p=P)
    yT_r = yT.ap().rearrange("(mc p) t -> p mc t", p=P)

    const = ctx.enter_context(tc.tile_pool(name="const", bufs=1))
    qkvp = ctx.enter_context(tc.tile_pool(name="qkvp", bufs=1))
    outp = ctx.enter_context(tc.tile_pool(name="outp", bufs=1))
    psA = ctx.enter_context(tc.tile_pool(name="psA", bufs=2, space="PSUM"))
    psO = ctx.enter_context(tc.tile_pool(name="psO", bufs=1, space="PSUM"))
    psY = ctx.enter_context(tc.tile_pool(name="psY", bufs=2, space="PSUM"))

    # ---- constants ----
    bqk_sb = const.tile([P, 4], F32)     # 32*bias, per 128-ch group column
    nc.sync.dma_start(out=bqk_sb, in_=bqk.ap().rearrange("(c p) -> p c", p=P))
    # touch Exp once so ACT's table set loads during the input DMA instead of
    # stalling the first real softmax exp mid-pipeline (~2.7us)
    warm = const.tile([1, 1], F32)
    nc.scalar.activation(out=warm, in_=bqk_sb[0:1, 0:1], func=FT.Exp)
    bv_sb = const.tile([1, CH], F32)
    nc.sync.dma_start(out=bv_sb, in_=bv.ap())
    bvb = const.tile([P, CH], F32)
    nc.gpsimd.partition_broadcast(out_ap=bvb, in_ap=bv_sb)
    # schraudolph constants as fp32 APs (immediates may lose precision)
    exc_sb = const.tile([P, 2], F32)
    nc.vector.memset(exc_sb[:, 0:1], EXA)
    nc.vector.memset(exc_sb[:, 1:2], EXB)
    wp_sb = const.tile([P, 2, D], BF16)
    for cc in range(2):
        nc.sync.dma_start(out=wp_sb[:, cc, :], in_=wp_r[:, cc, :])

    # ---- persistent activations ----
    # q/k score operands, fp8 hi/lo layout. Per head-pair a (= hp):
    #   ksc cols: 0=[kh_e;kl_e] 1=[kh_e;0] 2=[kl_o;kh_o] 3=[0;kh_o]
    #   qsc cols: 0=[qh_e;qh_e] 1=[ql_e;ql_o] 2=[qh_o;qh_o] 3=[ql_e;ql_o]
    # head b in pair: lhsT = ksc[:, a, 2b:2b+2, j], rhs = qsc[:, a, 2b:2b+2, i]
    ksc = qkvp.tile([P, 2, 4, T], FP8)
    qsc = qkvp.tile([P, 2, 4, T], FP8)
    # staged q/k hi/lo in qkv-psum partition order (head-even 0:64, odd 64:128)
    stg = ctx.enter_context(tc.tile_pool(name="stg", bufs=1))
    qh_st = stg.tile([P, 2, T], FP8)
    ql_st = stg.tile([P, 2, T], FP8)
    kh_st = stg.tile([P, 2, T], FP8)
    kl_st = stg.tile([P, 2, T], FP8)
    v_sb = qkvp.tile([P, TJ, HPC * 65], BF16)      # v_aug  [T, 4*(64+1)]
    at_sb = outp.tile([P, 2, T], BF16)             # attn_outT [ch, T]

    # zero the k-side dead halves once (read by every scores matmul; the
    # rhs there can be junk only because these lhsT halves are exactly 0)
    for a in range(2):
        nc.gpsimd.memset(ksc[64:128, a, 1, :], 0)
        nc.gpsimd.memset(ksc[0:64, a, 3, :], 0)

    v4 = v_sb.rearrange("p j (h u) -> p j h u", u=65)
    # ones column per head (softmax denominators fall out of the PV matmul)
    nc.scalar.activation(
        out=v4[:, :, :, 64],
        in_=wp_sb[:, 0, 0:TJ * HPC].rearrange("p (j h) -> p j h", h=HPC),
        func=FT.Copy,
        bias=1.0,
        scale=0.0,
    )

    for _rep in range(repeat):
      with ExitStack() as rctx:
            xw = rctx.enter_context(tc.tile_pool(name="xw", bufs=1))
            work = rctx.enter_context(tc.tile_pool(name="work", bufs=3))
            xh_sb = xw.tile([P, KC, 2, T], FP8)
            xl_sb = xw.tile([P, KC, 2, T], FP8)
            wqk_sb = xw.tile([P, KC, 2, 2, 2 * CH], FP8)
            wv_sb = xw.tile([P, KC, 2, 2, CH], FP8)
            # chunk-major: the qk chains can start as soon as chunk 0 lands
            for kc in range(KC):
                nc.sync.dma_start(out=wqk_sb[:, kc], in_=wqk_r[:, kc])
                nc.sync.dma_start(out=xh_sb[:, kc], in_=xh_r[:, kc])
            for kc in range(KC):
                nc.sync.dma_start(out=xl_sb[:, kc], in_=xl_r[:, kc])
            nc.sync.dma_start(out=wv_sb, in_=wv_r)

            def qkv_ps(n, front=False):
                # deferred (mid-attention) chains must NOT rotate the 'st'
                # tag: scores tiles live there, and sharing its 2 buffers
                # serializes scores against chain completions. 'py' is free
                # until the projections start.
                if front and n % 2 == 1:
                    return psA.tile([P, 512], F32, name="ps", tag="st", bufs=2)
                return psY.tile([P, 512], F32, name="ps", tag="py", bufs=2)

            nmm = 0
            TERMS = ((0, 0), (1, 0), (0, 1))   # (w hi/lo, x hi/lo)

            # ---- q/k chain + stage passes for 128-ch group cc ----------
            def qk_chain(cc, ic, act_hi=False):
                nonlocal nmm
                ps = qkv_ps(nmm, front=True)
                nmm += 1
                first = True
                for wi, xi in TERMS:
                    xt = xh_sb if xi == 0 else xl_sb
                    for kc in range(KC):
                        nc.tensor.matmul(
                            ps,
                            lhsT=wqk_sb[:, kc, :, wi, cc * P:(cc + 1) * P],
                            rhs=xt[:, kc, :, ic * 512:(ic + 1) * 512],
                            start=first,
                            stop=(kc == KC - 1 and (wi, xi) == TERMS[-1]),
                            perf_mode=DR,
                        )
                        first = False
                a = cc % 2
                hi_st, lo_st = (qh_st, ql_st) if cc < 2 else (kh_st, kl_st)
                cols = slice(ic * 512, (ic + 1) * 512)
                qf = work.tile([P, 512], F32, name="qf", bufs=3)
                nc.vector.tensor_scalar(
                    out=qf, in0=ps, scalar1=bqk_sb[:, cc:cc + 1],
                    scalar2=1.0 / W_SCALE, op0=OP.add, op1=OP.mult)
                if act_hi:
                    nc.scalar.copy(out=hi_st[:, a, cols], in_=qf)
                else:
                    nc.vector.tensor_copy(out=hi_st[:, a, cols], in_=qf)
                nc.vector.scalar_tensor_tensor(
                    out=lo_st[:, a, cols], in0=hi_st[:, a, cols], scalar=-1.0,
                    in1=qf, op0=OP.mult, op1=OP.add)

            # ---- SBUF->SBUF DMAs placing the hi/lo halves for pair a ----
            # duplicate destinations ride stride-0 broadcast APs so each
            # placement is ONE DMA (HWDGE/SWDGE generation is the scarce
            # resource, not transfer bandwidth)
            def stage_dmas_k(a, eng, lo=0, hi=T):
                c = slice(lo, hi)
                w = hi - lo
                eng.dma_start(
                    out=ksc[0:64, a, 0:2, c],
                    in_=kh_st[0:64, a, c].unsqueeze(1).broadcast_to([64, 2, w]))
                eng.dma_start(
                    out=ksc[64:128, a, 2:4, c],
                    in_=kh_st[64:128, a, c].unsqueeze(1).broadcast_to([64, 2, w]))
                eng.dma_start(out=ksc[64:128, a, 0, c], in_=kl_st[0:64, a, c])
                eng.dma_start(out=ksc[0:64, a, 2, c], in_=kl_st[64:128, a, c])

            def stage_dmas_q(a, eng, lo=0, hi=T):
                c = slice(lo, hi)
                w = hi - lo
                eng.dma_start(
                    out=qsc[0:64, a, 0, c].unsqueeze(1),
                    in_=qh_st[0:64, a, c].unsqueeze(1))
                eng.dma_start(out=qsc[64:128, a, 0, c], in_=qh_st[0:64, a, c])
                eng.dma_start(out=qsc[0:64, a, 2, c], in_=qh_st[64:128, a, c])
                eng.dma_start(out=qsc[64:128, a, 2, c], in_=qh_st[64:128, a, c])
                eng.dma_start(
                    out=qsc[:, a, 1:4:2, c],
                    in_=ql_st[:, a, c].unsqueeze(1).broadcast_to([P, 2, w]))

            # a=0 chains run serially up front (attention's first blocks
            # need them); a=1 chains + v chains dribble into the pipeline.
            # k chains first so the k stage-DMAs overlap the q chains; the
            # a=0 stage DMAs ride the otherwise-idle ACT queue.
            qk_chain(2, 0, act_hi=True)
            stage_dmas_k(0, nc.sync, 0, 512)
            qk_chain(0, 0, act_hi=True)
            stage_dmas_q(0, nc.sync, 0, 512)
            for ic in range(1, TI):
                qk_chain(2, ic, act_hi=True)
            stage_dmas_k(0, nc.gpsimd, 512, T)
            for ic in range(1, TI):
                qk_chain(0, ic, act_hi=True)
            stage_dmas_q(0, nc.sync, 512, T)

            q_v = []

            def defer_v_chain(jc):
                def f():
                    nonlocal nmm
                    ps = qkv_ps(nmm)[:, :CH]
                    nmm += 1
                    first = True
                    for wi, xi in TERMS:
                        xt = xh_sb if xi == 0 else xl_sb
                        for kc in range(KC):
                            nc.tensor.matmul(
                                ps,
                                lhsT=xt[:, kc, :, jc * P:(jc + 1) * P],
                                rhs=wv_sb[:, kc, :, wi, :],
                                start=first,
                                stop=(kc == KC - 1 and (wi, xi) == TERMS[-1]),
                                perf_mode=DR,
                            )
                            first = False
                    nc.vector.scalar_tensor_tensor(
                        out=v4[:, jc, :, 0:64],
                        in0=ps.rearrange("p (h u) -> p h u", u=64),
                        scalar=1.0 / W_SCALE,
                        in1=bvb.rearrange("p (h u) -> p h u", u=64),
                        op0=OP.mult, op1=OP.add)
                return f

            for jc in range(TJ):
                q_v.append(defer_v_chain(jc))

            # deferred a=1 q/k chains (one closure per matmul-third)
            q_qkv = []

            def defer_qk_chain(cc, ic):
                state = {}

                def mk(kc, ti):
                    def f():
                        nonlocal nmm
                        if kc == 0 and ti == 0:
                            state["ps"] = qkv_ps(nmm)
                            nmm += 1
                        ps = state["ps"]
                        wi, xi = TERMS[ti]
                        xt = xh_sb if xi == 0 else xl_sb
                        nc.tensor.matmul(
                            ps,
                            lhsT=wqk_sb[:, kc, :, wi, cc * P:(cc + 1) * P],
                            rhs=xt[:, kc, :, ic * 512:(ic + 1) * 512],
                            start=(kc == 0 and ti == 0),
                            stop=(kc == KC - 1 and ti == 2),
                            perf_mode=DR,
                        )
                        if kc == KC - 1 and ti == 2:
                            a = cc % 2
                            hi_st, lo_st = (qh_st, ql_st) if cc < 2 else (kh_st, kl_st)
                            cols = slice(ic * 512, (ic + 1) * 512)
                            qf = work.tile([P, 512], F32, name="qf", bufs=3)
                            nc.vector.tensor_scalar(
                                out=qf, in0=ps, scalar1=bqk_sb[:, cc:cc + 1],
                                scalar2=1.0 / W_SCALE, op0=OP.add, op1=OP.mult)
                            nc.vector.tensor_copy(out=hi_st[:, a, cols], in_=qf)
                            nc.vector.scalar_tensor_tensor(
                                out=lo_st[:, a, cols], in0=hi_st[:, a, cols],
                                scalar=-1.0, in1=qf, op0=OP.mult, op1=OP.add)
                    return f

                q_qkv.extend(mk(kc, ti) for kc in range(KC) for ti in range(3))

            # a=1: k chains, k DMAs, q chains, q DMAs — all dribbled in order
            for ic in range(TI):
                defer_qk_chain(3, ic)
            q_qkv.append(lambda: stage_dmas_k(1, nc.sync))
            for ic in range(TI):
                defer_qk_chain(1, ic)
            q_qkv.append(lambda: stage_dmas_q(1, nc.sync))

            # ---- phase 2: attention + out-proj, one flat software pipeline
            LAG = lag
            blocks = [(ic, a) for a in range(2) for ic in range(TI)]
            steps = [(bi, jc) for bi in range(len(blocks)) for jc in range(TJ)]
            po_of = {}
            pt_of = {}
            q_proj = []

            def emit_scores_exp(bi, jc):
                ic, a = blocks[bi]
                st = psA.tile([P, 2, 512], F32, name="st", tag="st", bufs=2)
                for b in range(2):
                    # S.T[j, i] = kh.qh + kl.qh + kh.ql via one DoubleRow MM
                    nc.tensor.matmul(
                        st[:, b, :],
                        lhsT=ksc[:, a, 2 * b:2 * b + 2, jc * P:(jc + 1) * P],
                        rhs=qsc[:, a, 2 * b:2 * b + 2, ic * 512:(ic + 1) * 512],
                        start=True,
                        stop=True,
                        perf_mode=DR,
                    )
                pt = work.tile([P, 2, 512], BF16, name="pt", bufs=LAG + 1)
                eng = OFF_STEPS.get(jc)
                if eng is not None:
                    tf = work.tile([P, 2, 512], F32, name="tf", bufs=2)
                    nc.vector.tensor_scalar(
                        out=tf, in0=st,
                        scalar1=exc_sb[:, 0:1], scalar2=exc_sb[:, 1:2],
                        op0=OP.mult, op1=OP.add)
                    nc.vector.tensor_copy(out=pt.bitcast(I16), in_=tf)
                else:
                    nc.scalar.activation(out=pt, in_=st, func=FT.Exp, scale=SCALE)
                pt_of[(bi, jc)] = pt

            def emit_pv(bi, jc):
                ic, a = blocks[bi]
                if jc == 0:
                    po_of[bi] = [
                        psO.tile([P, 512], F32, name=f"po{b}", tag=f"po{b}",
                                 bufs=1)
                        for b in range(2)
                    ]
                pt = pt_of.pop((bi, jc))
                for b in range(2):
                    nc.tensor.matmul(
                        po_of[bi][b][0:65, :],
                        lhsT=v4[:, jc, a * 2 + b, :],
                        rhs=pt[:, b, :],
                        start=(jc == 0),
                        stop=(jc == TJ - 1),
                    )

            def emit_norm(bi):
                ic, a = blocks[bi]
                po = po_of.pop(bi)
                for b in range(2):
                    # one copy of rows 0:65 frees the PSUM bank immediately;
                    # the rest of the chain runs off SBUF. (custom-DVE ops
                    # misread APs at partition offset 64 on HW -- stage the
                    # sums row into a base-0 tile before reciprocal.)
                    osb = work.tile([65, 512], F32, name="osb", bufs=2)
                    nc.vector.tensor_copy(out=osb, in_=po[b][0:65, :])
                    sums = work.tile([1, 512], F32, name="sums")
                    nc.gpsimd.tensor_copy(out=sums, in_=osb[64:65, :])
                    rr = work.tile([1, 512], F32, name="rr")
                    nc.vector.reciprocal_approx_fast(out=rr, in_=sums)
                    rb = work.tile([64, 512], F32, name="rb", bufs=2)
                    nc.gpsimd.partition_broadcast(out_ap=rb, in_ap=rr)
                    nc.vector.tensor_tensor(
                        out=at_sb[b * 64:(b + 1) * 64, a,
                                  ic * 512:(ic + 1) * 512],
                        in0=osb[0:64, :],
                        in1=rb,
                        op=OP.mult,
                    )

            def defer_proj(ic):
                def mk(mc):
                    def f():
                        py = psY.tile([P, 512], F32, name="py", tag="py",
                                      bufs=2)
                        for cc in range(2):
                            nc.tensor.matmul(
                                py,
                                lhsT=wp_sb[:, cc, mc * P:(mc + 1) * P],
                                rhs=at_sb[:, cc, ic * 512:(ic + 1) * 512],
                                start=(cc == 0), stop=(cc == 1),
                            )
                        yt = work.tile([P, 512], BF16, name="yt", bufs=6)
                        nc.vector.tensor_copy(out=yt, in_=py)
                        nc.sync.dma_start(
                            out=yT_r[:, mc, ic * 512:(ic + 1) * 512], in_=yt)
                    return f

                q_proj.extend(mk(mc) for mc in range(MC))

            # emission order per step: PV(n-LAG) + norm, then scores(n)
            # (waits ride the PE's 4-deep wait queue), then dribbles so the
            # first blocks' scores aren't queued behind 16 v-chains.
            for idx in range(len(steps)):
                if idx >= LAG:
                    bi, jc = steps[idx - LAG]
                    emit_pv(bi, jc)
                    if jc == TJ - 1:
                        emit_norm(bi)
                        ic, a = blocks[bi]
                        if a == 1:
                            defer_proj(ic)
                bi, jc = steps[idx]
                if blocks[bi][1] == 1:
                    # a=1 scores need the deferred chains + stage DMAs
                    while q_qkv:
                        q_qkv.pop(0)()
                emit_scores_exp(bi, jc)
                for _ in range(2):
                    if q_v:
                        q_v.pop(0)()
                    elif q_qkv:
                        q_qkv.pop(0)()
                        if q_qkv:
                            q_qkv.pop(0)()
                    elif q_proj:
                        q_proj.pop(0)()
            # tail: flush everything immediately (execution is dep-driven)
            for idx in range(len(steps), len(steps) + LAG):
                bi, jc = steps[idx - LAG]
                emit_pv(bi, jc)
                if jc == TJ - 1:
                    emit_norm(bi)
                    ic, a = blocks[bi]
                    if a == 1:
                        defer_proj(ic)
            while q_proj:
                q_proj.pop(0)()


def build_nc(T, repeat=1, **kw):
    nc = bacc.Bacc("TRN2", target_bir_lowering=False, debug=False)
    xh = nc.dram_tensor("xh", [D, T], FP8, kind="ExternalInput")
    xl = nc.dram_tensor("xl", [D, T], FP8, kind="ExternalInput")
    wqkhl = nc.dram_tensor("wqkhl", [D, 2, 2 * CH], FP8, kind="ExternalInput")
    wvhl = nc.dram_tensor("wvhl", [D, 2, CH], FP8, kind="ExternalInput")
    bqk = nc.dram_tensor("bqk", [2 * CH], F32, kind="ExternalInput")
    bv = nc.dram_tensor("bv", [1, CH], F32, kind="ExternalInput")
    wpT = nc.dram_tensor("wpT", [CH, D], BF16, kind="ExternalInput")
    yT = nc.dram_tensor("yT", [D, T], BF16, kind="ExternalOutput")
    with tile.TileContext(nc) as tc, ExitStack() as ctx:
        build_body(tc, ctx, T, xh, xl, wqkhl, wvhl, bqk, bv, wpT, yT,
                   repeat=repeat, **kw)
    nc.compile()
    return nc


E4NP = ml_dtypes.float8_e4m3


def _hilo_stack(a):
    h = np.ascontiguousarray(a).astype(E4NP)
    l = np.ascontiguousarray(a - h.astype(np.float32)).astype(E4NP)
    return np.ascontiguousarray(np.stack([h, l], axis=1))


def make_in_maps(x, w_attn, b_attn, w_proj):
    x = np.ascontiguousarray(np.asarray(x, dtype=np.float32))
    w_attn = np.asarray(w_attn, dtype=np.float32)
    b_attn = np.asarray(b_attn, dtype=np.float32)
    w_proj = np.asarray(w_proj, dtype=np.float32)
    in_maps = []
    for c in range(NCORES):
        b, g = divmod(c, 4)
        sl = slice(g * CH, (g + 1) * CH)
        wq, wk, wv = w_attn[0 * D:][sl], w_attn[1 * D:][sl], w_attn[2 * D:][sl]
        xh8 = np.ascontiguousarray(x[b].T).astype(E4NP)
        xl8 = np.ascontiguousarray(x[b].T - xh8.astype(np.float32)).astype(E4NP)
        in_maps.append({
            "xh": xh8, "xl": xl8,
            "wqkhl": _hilo_stack(np.concatenate([wq, wk], 0).T * W_SCALE),
            "wvhl": _hilo_stack(wv.T * W_SCALE),
            "bqk": np.ascontiguousarray(
                np.concatenate([b_attn[0 * D:][sl], b_attn[1 * D:][sl]])
                * W_SCALE),
            "bv": np.ascontiguousarray(b_attn[2 * D:][sl][None, :]),
            "wpT": np.ascontiguousarray(
                w_proj[:, sl].T.astype(ml_dtypes.bfloat16)),
        })
    return in_maps


LAG_DEFAULT = 12

_NC_CACHE = {}


def _get_nc(T):
    key = (T, LAG_DEFAULT)
    if key not in _NC_CACHE:
        _NC_CACHE[key] = build_nc(T, lag=LAG_DEFAULT)
    return _NC_CACHE[key]


def run(x, w_attn, b_attn, w_proj, b_proj, trace=False, **hw_kwargs):
    T = np.asarray(x).shape[1]
    nc = _get_nc(T)
    in_maps = make_in_maps(x, w_attn, b_attn, w_proj)
    res = run_bass_kernel_spmd(
        nc, in_maps, core_ids=list(range(NCORES)), trace=trace, **hw_kwargs
    )
    y = np.zeros((B, T, D), dtype=np.float32)
    for c in range(NCORES):
        y[c // 4] += res.results[c]["yT"].T.astype(np.float32)
    y += np.asarray(b_proj, dtype=np.float32)
    return y, res


def kernel(x, w_attn, b_attn, w_proj, b_proj):
    y, _ = run(x, w_attn, b_attn, w_proj, b_proj)
    return y


# revision 30
# speedup vs baseline: 1.0308x; 1.0308x over previous
"""Fused multi-head attention block (qkv proj + softmax(QK^T)V + out proj)
for Trainium2, SPMD across 8 NeuronCores.

Sharding: 8 cores = 2 batches x 4 head-groups (4 heads/core, data parallel on
B, tensor parallel on heads). Each core computes its 4 heads end-to-end plus a
row-parallel slice of the output projection; the 4 partial y's per batch are
summed on the host (with b_proj added once).

v3 dataflow (per core, fp8-DoubleRow hi/lo decomposition):
  - QKV: x and the (host-x32-scaled) weights are shipped as fp8e4 hi/lo
    pairs; each projection accumulates three DoubleRow chains
    (xh*wh + xh*wl + xl*wh) in PSUM, then an affine pass applies
    (psum + 32*bias)/32. q/k come out [ch, T]; v comes out [T, ch] with a
    ones column per head (row sums -> softmax denominators).
  - scores: per head, ONE DoubleRow matmul computes kh.qh + kl.qh + kh.ql
    (K = 2x128: slot0 = [kh;kl]x[qh;qh], slot1 = [kh;0]x[ql;*]). The fp8
    hi/lo q/k operand tiles are built by DVE cast passes into staged
    [2-head, T] tiles plus SBUF->SBUF DMAs that place/duplicate the 64-row
    halves (issued on the ACT queue for a=0, SP for a=1).
  - softmax: P.T = exp(S.T * 0.125) out of PSUM, bf16; most steps on ACT
    (native Exp), 3 of 16 per block on DVE/Pool via a Schraudolph bit-trick
    (t = s*a + b -> int16 -> bitcast bf16). Normalization deferred via the
    ones column in v; no max subtraction needed at these score magnitudes.
  - PV: O.T_aug[65, i] += v_aug[jc].T @ P.T[jc], bf16 matmuls.
  - normalize: one [65,512] copy frees the PSUM bank, then recip/broadcast/
    multiply off SBUF -> attn.T bf16.
  - proj: split by K-half. The cc=0 half (a=0 heads) runs right after the
    a=0 norm and is staged to SBUF; the cc=1 half runs after the a=1 norm
    and a tensor_tensor add emits yT bf16 partials (keeps the tail short).
"""

from contextlib import ExitStack

import ml_dtypes
import numpy as np

import concourse.bass as bass
import concourse.mybir as mybir
import concourse.tile as tile
from concourse import bacc
from concourse.bass_utils import run_bass_kernel_spmd

F32 = mybir.dt.float32
BF16 = mybir.dt.bfloat16
FP8 = mybir.dt.float8e4
I16 = mybir.dt.int16
FT = mybir.ActivationFunctionType
OP = mybir.AluOpType
DR = mybir.MatmulPerfMode.DoubleRow

B, D = 2, 1024
H, HD = 16, 64
NCORES = 8
HPC = 4                # heads per core
CH = HPC * HD          # 256 q/k/v channels per core
P = 128
W_SCALE = 32.0         # host-side weight scale before fp8 hi/lo split
SCALE = 1.0 / float(np.sqrt(HD))
# Schraudolph bf16 exp: floor(s*EXA + EXB) as int16, bitcast bf16 ~= exp(s*SCALE)
EXA = 128.0 * float(np.log2(np.e)) * SCALE
EXB = 127.0 * 128.0 - 7.5 + 0.5   # c=-7.5 calibrated; +0.5 turns floor into round
OFF_STEPS = {}   # schraudolph exp on HW loses precision; keep exp on ACT


def build_body(tc, ctx, T, xh, xl, wqkhl, wvhl, bqk, bv, wpT, yT,
               repeat=1, lag=8):
    nc = tc.nc
    TI = T // 512          # 512-wide i (query) chunks
    TJ = T // P            # 128-wide j (key) chunks
    KC = 4                 # DoubleRow contraction chunks (K=1024 as 4x256)
    MC = D // P            # 8 output-row chunks

    xh_r = xh.ap().rearrange("(c s p) t -> p c s t", p=P, s=2)
    xl_r = xl.ap().rearrange("(c s p) t -> p c s t", p=P, s=2)
    wqk_r = wqkhl.ap().rearrange("(c s p) h m -> p c s h m", p=P, s=2)
    wv_r = wvhl.ap().rearrange("(c s p) h m -> p c s h m", p=P, s=2)
    wp_r = wpT.ap().rearrange("(cc p) m -> p cc m", p=P)
    yT_r = yT.ap().rearrange("(mc p) t -> p mc t", p=P)

    const = ctx.enter_context(tc.tile_pool(name="const", bufs=1))
    qkvp = ctx.enter_context(tc.tile_pool(name="qkvp", bufs=1))
    outp = ctx.enter_context(tc.tile_pool(name="outp", bufs=1))
    psA = ctx.enter_context(tc.tile_pool(name="psA", bufs=2, space="PSUM"))
    psO = ctx.enter_context(tc.tile_pool(name="psO", bufs=1, space="PSUM"))
    psY = ctx.enter_context(tc.tile_pool(name="psY", bufs=2, space="PSUM"))

    # ---- constants ----
    bqk_sb = const.tile([P, 4], F32)     # 32*bias, per 128-ch group column
    nc.sync.dma_start(out=bqk_sb, in_=bqk.ap().rearrange("(c p) -> p c", p=P))
    # touch Exp once so ACT's table set loads during the input DMA instead of
    # stalling the first real softmax exp mid-pipeline (~2.7us)
    warm = const.tile([1, 1], F32)
    nc.scalar.activation(out=warm, in_=bqk_sb[0:1, 0:1], func=FT.Exp)
    bv_sb = const.tile([1, CH], F32)
    nc.sync.dma_start(out=bv_sb, in_=bv.ap())
    bvb = const.tile([P, CH], F32)
    nc.gpsimd.partition_broadcast(out_ap=bvb, in_ap=bv_sb)
    # schraudolph constants as fp32 APs (immediates may lose precision)
    exc_sb = const.tile([P, 2], F32)
    nc.vector.memset(exc_sb[:, 0:1], EXA)
    nc.vector.memset(exc_sb[:, 1:2], EXB)
    wp_sb = const.tile([P, 2, D], BF16)

    # ---- persistent activations ----
    # q/k score operands, fp8 hi/lo layout. Per head-pair a (= hp):
    #   ksc cols: 0=[kh_e;kl_e] 1=[kh_e;0] 2=[kl_o;kh_o] 3=[0;kh_o]
    #   qsc cols: 0=[qh_e;qh_e] 1=[ql_e;ql_o] 2=[qh_o;qh_o] 3=[ql_e;ql_o]
    # head b in pair: lhsT = ksc[:, a, 2b:2b+2, j], rhs = qsc[:, a, 2b:2b+2, i]
    ksc = qkvp.tile([P, 2, 4, T], FP8)
    qsc = qkvp.tile([P, 2, 4, T], FP8)
    # staged q/k hi/lo in qkv-psum partition order (head-even 0:64, odd 64:128)
    stg = ctx.enter_context(tc.tile_pool(name="stg", bufs=1))
    qh_st = stg.tile([P, 2, T], FP8)
    ql_st = stg.tile([P, 2, T], FP8)
    kh_st = stg.tile([P, 2, T], FP8)
    kl_st = stg.tile([P, 2, T], FP8)
    v_sb = qkvp.tile([P, TJ, HPC * 65], BF16)      # v_aug  [T, 4*(64+1)]
    at_sb = outp.tile([P, 2, T], BF16)             # attn_outT [ch, T]

    # zero the k-side dead halves once (read by every scores matmul; the
    # rhs there can be junk only because these lhsT halves are exactly 0)
    for a in range(2):
        nc.gpsimd.memset(ksc[64:128, a, 1, :], 0)
        nc.gpsimd.memset(ksc[0:64, a, 3, :], 0)

    v4 = v_sb.rearrange("p j (h u) -> p j h u", u=65)
    # ones column per head (softmax denominators fall out of the PV
    # matmul). bvb is the input-source: it is initialized by the early bv
    # DMA+broadcast (wp_sb now loads late, after the x transfers).
    nc.scalar.activation(
        out=v4[:, :, :, 64],
        in_=bvb[:, 0:TJ * HPC].rearrange("p (j h) -> p j h", h=HPC),
        func=FT.Copy,
        bias=1.0,
        scale=0.0,
    )

    for _rep in range(repeat):
      with ExitStack() as rctx:
            xw = rctx.enter_context(tc.tile_pool(name="xw", bufs=1))
            work = rctx.enter_context(tc.tile_pool(name="work", bufs=3))
            xh_sb = xw.tile([P, KC, 2, T], FP8)
            xl_sb = xw.tile([P, KC, 2, T], FP8)
            wqk_sb = xw.tile([P, KC, 2, 2, 2 * CH], FP8)
            wv_sb = xw.tile([P, KC, 2, 2, CH], FP8)
            # chunk-major: the qk chains can start as soon as chunk 0 lands
            for kc in range(KC):
                nc.sync.dma_start(out=wqk_sb[:, kc], in_=wqk_r[:, kc])
                nc.sync.dma_start(out=xh_sb[:, kc], in_=xh_r[:, kc])
            for kc in range(KC):
                nc.sync.dma_start(out=xl_sb[:, kc], in_=xl_r[:, kc])
            # late inputs: not needed until the attention phase starts
            for cc in range(2):
                nc.sync.dma_start(out=wp_sb[:, cc, :], in_=wp_r[:, cc, :])
            nc.sync.dma_start(out=wv_sb, in_=wv_r)

            def qkv_ps(n, front=False):
                # deferred (mid-attention) chains must NOT rotate the 'st'
                # tag: scores tiles live there, and sharing its 2 buffers
                # serializes scores against chain completions. 'py' is free
                # until the projections start.
                if front and n % 2 == 1:
                    return psA.tile([P, 512], F32, name="ps", tag="st", bufs=2)
                return psY.tile([P, 512], F32, name="ps", tag="py", bufs=2)

            nmm = 0
            TERMS = ((0, 0), (1, 0), (0, 1))   # (w hi/lo, x hi/lo)

            # ---- q/k chain + stage passes for 128-ch group cc ----------
            def qk_chain(cc, ic, act_hi=False):
                nonlocal nmm
                ps = qkv_ps(nmm, front=True)
                nmm += 1
                first = True
                for wi, xi in TERMS:
                    xt = xh_sb if xi == 0 else xl_sb
                    for kc in range(KC):
                        nc.tensor.matmul(
                            ps,
                            lhsT=wqk_sb[:, kc, :, wi, cc * P:(cc + 1) * P],
                            rhs=xt[:, kc, :, ic * 512:(ic + 1) * 512],
                            start=first,
                            stop=(kc == KC - 1 and (wi, xi) == TERMS[-1]),
                            perf_mode=DR,
                        )
                        first = False
                a = cc % 2
                hi_st, lo_st = (qh_st, ql_st) if cc < 2 else (kh_st, kl_st)
                cols = slice(ic * 512, (ic + 1) * 512)
                qf = work.tile([P, 512], F32, name="qf", bufs=3)
                nc.vector.tensor_scalar(
                    out=qf, in0=ps, scalar1=bqk_sb[:, cc:cc + 1],
                    scalar2=1.0 / W_SCALE, op0=OP.add, op1=OP.mult)
                if act_hi:
                    nc.scalar.copy(out=hi_st[:, a, cols], in_=qf)
                else:
                    nc.vector.tensor_copy(out=hi_st[:, a, cols], in_=qf)
                nc.vector.scalar_tensor_tensor(
                    out=lo_st[:, a, cols], in0=hi_st[:, a, cols], scalar=-1.0,
                    in1=qf, op0=OP.mult, op1=OP.add)

            # ---- SBUF->SBUF DMAs placing the hi/lo halves for pair a ----
            # duplicate destinations ride stride-0 broadcast APs so each
            # placement is ONE DMA (HWDGE/SWDGE generation is the scarce
            # resource, not transfer bandwidth)
            def stage_dmas_k(a, eng, lo=0, hi=T):
                c = slice(lo, hi)
                w = hi - lo
                eng.dma_start(
                    out=ksc[0:64, a, 0:2, c],
                    in_=kh_st[0:64, a, c].unsqueeze(1).broadcast_to([64, 2, w]))
                eng.dma_start(
                    out=ksc[64:128, a, 2:4, c],
                    in_=kh_st[64:128, a, c].unsqueeze(1).broadcast_to([64, 2, w]))
                eng.dma_start(out=ksc[64:128, a, 0, c], in_=kl_st[0:64, a, c])
                eng.dma_start(out=ksc[0:64, a, 2, c], in_=kl_st[64:128, a, c])

            def stage_dmas_q(a, eng, lo=0, hi=T):
                c = slice(lo, hi)
                w = hi - lo
                eng.dma_start(
                    out=qsc[0:64, a, 0, c].unsqueeze(1),
                    in_=qh_st[0:64, a, c].unsqueeze(1))
                eng.dma_start(out=qsc[64:128, a, 0, c], in_=qh_st[0:64, a, c])
                eng.dma_start(out=qsc[0:64, a, 2, c], in_=qh_st[64:128, a, c])
                eng.dma_start(out=qsc[64:128, a, 2, c], in_=qh_st[64:128, a, c])
                eng.dma_start(
                    out=qsc[:, a, 1:4:2, c],
                    in_=ql_st[:, a, c].unsqueeze(1).broadcast_to([P, 2, w]))

            # a=0 chains run serially up front (attention's first blocks
            # need them); a=1 chains + v chains dribble into the pipeline.
            # k chains first so the k stage-DMAs overlap the q chains; the
            # a=0 stage DMAs ride the otherwise-idle ACT queue.
            qk_chain(2, 0, act_hi=True)
            stage_dmas_k(0, nc.sync, 0, 512)
            qk_chain(0, 0, act_hi=True)
            stage_dmas_q(0, nc.sync, 0, 512)
            for ic in range(1, TI):
                qk_chain(2, ic, act_hi=True)
            stage_dmas_k(0, nc.gpsimd, 512, T)
            for ic in range(1, TI):
                qk_chain(0, ic, act_hi=True)
            stage_dmas_q(0, nc.sync, 512, T)

            q_v = []

            def defer_v_chain(jc):
                def f():
                    nonlocal nmm
                    ps = qkv_ps(nmm)[:, :CH]
                    nmm += 1
                    first = True
                    for wi, xi in TERMS:
                        xt = xh_sb if xi == 0 else xl_sb
                        for kc in range(KC):
                            nc.tensor.matmul(
                                ps,
                                lhsT=xt[:, kc, :, jc * P:(jc + 1) * P],
                                rhs=wv_sb[:, kc, :, wi, :],
                                start=first,
                                stop=(kc == KC - 1 and (wi, xi) == TERMS[-1]),
                                perf_mode=DR,
                            )
                            first = False
                    nc.vector.scalar_tensor_tensor(
                        out=v4[:, jc, :, 0:64],
                        in0=ps.rearrange("p (h u) -> p h u", u=64),
                        scalar=1.0 / W_SCALE,
                        in1=bvb.rearrange("p (h u) -> p h u", u=64),
                        op0=OP.mult, op1=OP.add)
                return f

            for jc in range(TJ):
                q_v.append(defer_v_chain(jc))

            # deferred a=1 q/k chains (one closure per matmul-third)
            q_qkv = []

            def defer_qk_chain(cc, ic):
                state = {}

                def mk(kc, ti):
                    def f():
                        nonlocal nmm
                        if kc == 0 and ti == 0:
                            state["ps"] = qkv_ps(nmm)
                            nmm += 1
                        ps = state["ps"]
                        wi, xi = TERMS[ti]
                        xt = xh_sb if xi == 0 else xl_sb
                        nc.tensor.matmul(
                            ps,
                            lhsT=wqk_sb[:, kc, :, wi, cc * P:(cc + 1) * P],
                            rhs=xt[:, kc, :, ic * 512:(ic + 1) * 512],
                            start=(kc == 0 and ti == 0),
                            stop=(kc == KC - 1 and ti == 2),
                            perf_mode=DR,
                        )
                        if kc == KC - 1 and ti == 2:
                            a = cc % 2
                            hi_st, lo_st = (qh_st, ql_st) if cc < 2 else (kh_st, kl_st)
                            cols = slice(ic * 512, (ic + 1) * 512)
                            qf = work.tile([P, 512], F32, name="qf", bufs=3)
                            nc.vector.tensor_scalar(
                                out=qf, in0=ps, scalar1=bqk_sb[:, cc:cc + 1],
                                scalar2=1.0 / W_SCALE, op0=OP.add, op1=OP.mult)
                            nc.vector.tensor_copy(out=hi_st[:, a, cols], in_=qf)
                            nc.vector.scalar_tensor_tensor(
                                out=lo_st[:, a, cols], in0=hi_st[:, a, cols],
                                scalar=-1.0, in1=qf, op0=OP.mult, op1=OP.add)
                    return f

                q_qkv.extend(mk(kc, ti) for kc in range(KC) for ti in range(3))

            # a=1: k chains, k DMAs, q chains, q DMAs — all dribbled in order
            for ic in range(TI):
                defer_qk_chain(3, ic)
            q_qkv.append(lambda: stage_dmas_k(1, nc.sync))
            for ic in range(TI):
                defer_qk_chain(1, ic)
            q_qkv.append(lambda: stage_dmas_q(1, nc.sync))

            # ---- phase 2: attention + out-proj, one flat software pipeline
            LAG = lag
            blocks = [(ic, a) for a in range(2) for ic in range(TI)]
            steps = [(bi, jc) for bi in range(len(blocks)) for jc in range(TJ)]
            po_of = {}
            pt_of = {}
            q_proj = []

            def emit_scores_exp(bi, jc):
                ic, a = blocks[bi]
                st = psA.tile([P, 2, 512], F32, name="st", tag="st", bufs=2)
                for b in range(2):
                    # S.T[j, i] = kh.qh + kl.qh + kh.ql via one DoubleRow MM
                    nc.tensor.matmul(
                        st[:, b, :],
                        lhsT=ksc[:, a, 2 * b:2 * b + 2, jc * P:(jc + 1) * P],
                        rhs=qsc[:, a, 2 * b:2 * b + 2, ic * 512:(ic + 1) * 512],
                        start=True,
                        stop=True,
                        perf_mode=DR,
                    )
                pt = work.tile([P, 2, 512], BF16, name="pt", bufs=LAG + 1)
                eng = OFF_STEPS.get(jc)
                if eng is not None:
                    tf = work.tile([P, 2, 512], F32, name="tf", bufs=2)
                    nc.vector.tensor_scalar(
                        out=tf, in0=st,
                        scalar1=exc_sb[:, 0:1], scalar2=exc_sb[:, 1:2],
                        op0=OP.mult, op1=OP.add)
                    nc.vector.tensor_copy(out=pt.bitcast(I16), in_=tf)
                else:
                    nc.scalar.activation(out=pt, in_=st, func=FT.Exp, scale=SCALE)
                pt_of[(bi, jc)] = pt

            def emit_pv(bi, jc):
                ic, a = blocks[bi]
                if jc == 0:
                    po_of[bi] = [
                        psO.tile([P, 512], F32, name=f"po{b}", tag=f"po{b}",
                                 bufs=1)
                        for b in range(2)
                    ]
                pt = pt_of.pop((bi, jc))
                for b in range(2):
                    nc.tensor.matmul(
                        po_of[bi][b][0:65, :],
                        lhsT=v4[:, jc, a * 2 + b, :],
                        rhs=pt[:, b, :],
                        start=(jc == 0),
                        stop=(jc == TJ - 1),
                    )

            def emit_norm(bi):
                ic, a = blocks[bi]
                po = po_of.pop(bi)
                for b in range(2):
                    # one copy of rows 0:65 frees the PSUM bank immediately;
                    # the rest of the chain runs off SBUF. (custom-DVE ops
                    # misread APs at partition offset 64 on HW -- stage the
                    # sums row into a base-0 tile before reciprocal.)
                    osb = work.tile([65, 512], F32, name="osb", bufs=2)
                    nc.vector.tensor_copy(out=osb, in_=po[b][0:65, :])
                    sums = work.tile([1, 512], F32, name="sums")
                    nc.gpsimd.tensor_copy(out=sums, in_=osb[64:65, :])
                    rr = work.tile([1, 512], F32, name="rr")
                    nc.vector.reciprocal_approx_fast(out=rr, in_=sums)
                    rb = work.tile([64, 512], F32, name="rb", bufs=2)
                    nc.gpsimd.partition_broadcast(out_ap=rb, in_ap=rr)
                    nc.vector.tensor_tensor(
                        out=at_sb[b * 64:(b + 1) * 64, a,
                                  ic * 512:(ic + 1) * 512],
                        in0=osb[0:64, :],
                        in1=rb,
                        op=OP.mult,
                    )

            def defer_proj(ic):
                def mk(mc):
                    def f():
                        py = psY.tile([P, 512], F32, name="py", tag="py",
                                      bufs=2)
                        for cc in range(2):
                            nc.tensor.matmul(
                                py,
                                lhsT=wp_sb[:, cc, mc * P:(mc + 1) * P],
                                rhs=at_sb[:, cc, ic * 512:(ic + 1) * 512],
                                start=(cc == 0), stop=(cc == 1),
                            )
                        yt = work.tile([P, 512], BF16, name="yt", bufs=6)
                        nc.vector.tensor_copy(out=yt, in_=py)
                        nc.sync.dma_start(
                            out=yT_r[:, mc, ic * 512:(ic + 1) * 512], in_=yt)
                    return f

                q_proj.extend(mk(mc) for mc in range(MC))

            # emission order per step: PV(n-LAG) + norm, then scores(n)
            # (waits ride the PE's 4-deep wait queue), then dribbles so the
            # first blocks' scores aren't queued behind 16 v-chains.
            for idx in range(len(steps)):
                if idx >= LAG:
                    bi, jc = steps[idx - LAG]
                    emit_pv(bi, jc)
                    if jc == TJ - 1:
                        emit_norm(bi)
                        ic, a = blocks[bi]
                        if a == 1:
                            defer_proj(ic)
                bi, jc = steps[idx]
                if blocks[bi][1] == 1:
                    # a=1 scores need the deferred chains + stage DMAs
                    while q_qkv:
                        q_qkv.pop(0)()
                emit_scores_exp(bi, jc)
                for _ in range(2):
                    if q_v:
                        q_v.pop(0)()
                    elif q_qkv:
                        q_qkv.pop(0)()
                        if q_qkv:
                            q_qkv.pop(0)()
                    elif q_proj:
                        q_proj.pop(0)()
            # tail: flush everything immediately (execution is dep-driven)
            for idx in range(len(steps), len(steps) + LAG):
                bi, jc = steps[idx - LAG]
                emit_pv(bi, jc)
                if jc == TJ - 1:
                    emit_norm(bi)
                    ic, a = blocks[bi]
                    if a == 1:
                        defer_proj(ic)
            while q_proj:
                q_proj.pop(0)()


def build_nc(T, repeat=1, **kw):
    nc = bacc.Bacc("TRN2", target_bir_lowering=False, debug=False)
    xh = nc.dram_tensor("xh", [D, T], FP8, kind="ExternalInput")
    xl = nc.dram_tensor("xl", [D, T], FP8, kind="ExternalInput")
    wqkhl = nc.dram_tensor("wqkhl", [D, 2, 2 * CH], FP8, kind="ExternalInput")
    wvhl = nc.dram_tensor("wvhl", [D, 2, CH], FP8, kind="ExternalInput")
    bqk = nc.dram_tensor("bqk", [2 * CH], F32, kind="ExternalInput")
    bv = nc.dram_tensor("bv", [1, CH], F32, kind="ExternalInput")
    wpT = nc.dram_tensor("wpT", [CH, D], BF16, kind="ExternalInput")
    yT = nc.dram_tensor("yT", [D, T], BF16, kind="ExternalOutput")
    with tile.TileContext(nc) as tc, ExitStack() as ctx:
        build_body(tc, ctx, T, xh, xl, wqkhl, wvhl, bqk, bv, wpT, yT,
                   repeat=repeat, **kw)
    nc.compile()
    return nc


E4NP = ml_dtypes.float8_e4m3


def _hilo_stack(a):
    h = np.ascontiguousarray(a).astype(E4NP)
    l = np.ascontiguousarray(a - h.astype(np.float32)).astype(E4NP)
    return np.ascontiguousarray(np.stack([h, l], axis=1))


def make_in_maps(x, w_attn, b_attn, w_proj):
    x = np.ascontiguousarray(np.asarray(x, dtype=np.float32))
    w_attn = np.asarray(w_attn, dtype=np.float32)
    b_attn = np.asarray(b_attn, dtype=np.float32)
    w_proj = np.asarray(w_proj, dtype=np.float32)
    in_maps = []
    for c in range(NCORES):
        b, g = divmod(c, 4)
        sl = slice(g * CH, (g + 1) * CH)
        wq, wk, wv = w_attn[0 * D:][sl], w_attn[1 * D:][sl], w_attn[2 * D:][sl]
        xh8 = np.ascontiguousarray(x[b].T).astype(E4NP)
        xl8 = np.ascontiguousarray(x[b].T - xh8.astype(np.float32)).astype(E4NP)
        in_maps.append({
            "xh": xh8, "xl": xl8,
            "wqkhl": _hilo_stack(np.concatenate([wq, wk], 0).T * W_SCALE),
            "wvhl": _hilo_stack(wv.T * W_SCALE),
            "bqk": np.ascontiguousarray(
                np.concatenate([b_attn[0 * D:][sl], b_attn[1 * D:][sl]])
                * W_SCALE),
            "bv": np.ascontiguousarray(b_attn[2 * D:][sl][None, :]),
            "wpT": np.ascontiguousarray(
                w_proj[:, sl].T.astype(ml_dtypes.bfloat16)),
        })
    return in_maps


LAG_DEFAULT = 12

_NC_CACHE = {}


def _get_nc(T):
    key = (T, LAG_DEFAULT)
    if key not in _NC_CACHE:
        _NC_CACHE[key] = build_nc(T, lag=LAG_DEFAULT)
    return _NC_CACHE[key]


def run(x, w_attn, b_attn, w_proj, b_proj, trace=False, **hw_kwargs):
    T = np.asarray(x).shape[1]
    nc = _get_nc(T)
    in_maps = make_in_maps(x, w_attn, b_attn, w_proj)
    res = run_bass_kernel_spmd(
        nc, in_maps, core_ids=list(range(NCORES)), trace=trace, **hw_kwargs
    )
    y = np.zeros((B, T, D), dtype=np.float32)
    for c in range(NCORES):
        y[c // 4] += res.results[c]["yT"].T.astype(np.float32)
    y += np.asarray(b_proj, dtype=np.float32)
    return y, res


def kernel(x, w_attn, b_attn, w_proj, b_proj):
    y, _ = run(x, w_attn, b_attn, w_proj, b_proj)
    return y
